# revision 1
# baseline (speedup 1.0000x reference)
"""Bass/Trainium2 kernel for nn_Bert_coss (8-core data-parallel over batch).

Computation (per example):
  o1 = relu(X1 @ W.T + b)            [S, H]
  o2 = relu(X2 @ W.T + b)            [S, H]
  o1_doc, o2_doc = mean over S       [H]
  out = sigmoid(relu(concat(o1_doc, o2_doc) @ fd_w.T + fd_b) @ ff_w.T + ff_b)
  scores[s] = o1e[s] . o2_doc   (o1e = o1 ++ o1_doc row), s in 0..S
  att = softmax(scores); output rows 0..S-1 = att[0:S], row S = out.

Key algorithmic simplification: the reference's full [S+1,S+1] co-attention
einsum is only consumed through its last column, so only S+1 dot products
against o2_doc are needed.

Device-side layout: host pre-transposes X to [V, S] so the matmul contraction
dim (V) lands on SBUF partitions with no on-device transpose. o1 is produced
directly in [H, S] layout, which makes the doc-mean a free-axis reduction
(fused into the relu eviction via ACT accum_out) and the score computation a
K=H matvec on the TensorEngine.
"""

import sys

for _p in ("/opt/trn_rl_repo",):
    if _p not in sys.path:
        sys.path.append(_p)

import numpy as np
from contextlib import ExitStack

import concourse.bass as bass
import concourse.tile as tile
from concourse import bacc, mybir
from concourse import bass_utils

B, S, V, H = 64, 512, 768, 256
NCORES = 8
BL = B // NCORES        # examples per core
KV = V // 128           # contraction chunks for the mlp matmul
MH = H // 128           # output-partition chunks of H

F32 = mybir.dt.float32
F32R = mybir.dt.float32r
F16 = mybir.dt.float16
BF16 = mybir.dt.bfloat16
AF = mybir.ActivationFunctionType


def _build_kernel(tc):
    nc = tc.nc
    x1t = nc.dram_tensor("x1t", [BL, V, S], F16, kind="ExternalInput").ap()
    x2t = nc.dram_tensor("x2t", [BL, V, S], F16, kind="ExternalInput").ap()
    wt = nc.dram_tensor("wt", [V, H], F16, kind="ExternalInput").ap()
    mlp_b = nc.dram_tensor("mlp_b", [H, 1], F32, kind="ExternalInput").ap()
    fdwt = nc.dram_tensor("fdwt", [2 * H, H], F32, kind="ExternalInput").ap()
    fd_b = nc.dram_tensor("fd_b", [H, 1], F32, kind="ExternalInput").ap()
    ffwt = nc.dram_tensor("ffwt", [H, 1], F32, kind="ExternalInput").ap()
    ff_b = nc.dram_tensor("ff_b", [1, 1], F32, kind="ExternalInput").ap()
    out = nc.dram_tensor("out", [BL, S + 1], F32, kind="ExternalOutput").ap()

    with ExitStack() as ctx:
        const = ctx.enter_context(tc.tile_pool(name="const", bufs=1))

        # weight chunks as separate tiles so the k=0 matmul only depends on
        # the first small DMA; chunks beyond k=1 are issued interleaved with
        # the first X-chunk DMAs (DMA completions are FIFO per queue)
        wt_v = wt.rearrange("(k p) h -> p k h", p=128)
        wt_tiles = []
        for k in range(KV):
            wtk = const.tile([128, H], F16, tag=f"wt{k}")
            wt_tiles.append(wtk)

        def _wt_dma(k):
            # scalar-queue: weight completions don't gate the X stream's FIFO
            nc.scalar.dma_start(wt_tiles[k][:], wt_v[:, k, :])

        for k in range(KV):
            _wt_dma(k)
        mlpb_sb = const.tile([128, MH], F32)
        fdwt_sb = const.tile([128, 4 * H], F32)
        fdb_sb = const.tile([128, MH], F32)
        ffwt_sb = const.tile([128, MH], F32)
        ffb_sb = const.tile([1, 1], F32)
        nffb_sb = const.tile([1, 1], F32)
        expwarm = const.tile([1, 1], F32)
        # dummy Exp so the ACT table set loads during the DMA ramp instead of
        # on the end-of-kernel critical path
        nc.scalar.activation(expwarm[:], wt_tiles[0][0:1, 0:1], AF.Exp, scale=0.0)

        def _mlpb_dma():
            nc.scalar.dma_start(
                mlpb_sb[:].rearrange("p (m o) -> p m o", m=MH),
                mlp_b.rearrange("(m p) o -> p m o", p=128),
            )

        def _late_const_dmas():
            # parameters only needed by the end-of-kernel head
            nc.scalar.dma_start(
                fdwt_sb[:].rearrange("p (k h) -> p k h", k=4),
                fdwt.rearrange("(k p) h -> p k h", p=128),
            )
            nc.scalar.dma_start(
                fdb_sb[:].rearrange("p (m o) -> p m o", m=MH),
                fd_b.rearrange("(m p) o -> p m o", p=128),
            )
            nc.scalar.dma_start(
                ffwt_sb[:].rearrange("p (m o) -> p m o", m=MH),
                ffwt.rearrange("(m p) o -> p m o", p=128),
            )
            nc.scalar.dma_start(ffb_sb[:], ff_b[:, :])
            nc.vector.tensor_scalar_mul(nffb_sb[:], ffb_sb[:], -1.0)

        # doc-vector raw sums; column b*4 + kc, kc in (o1m0, o1m1, o2m0, o2m1)
        docs_all = const.tile([128, 4 * BL], F32)

        with ExitStack() as mctx:
            xpool = mctx.enter_context(tc.tile_pool(name="x", bufs=5))
            o1pool = mctx.enter_context(tc.tile_pool(name="o1", bufs=2))
            o2pool = mctx.enter_context(tc.tile_pool(name="o2", bufs=2))
            dpool = mctx.enter_context(tc.tile_pool(name="docs", bufs=2))
            apool = mctx.enter_context(tc.tile_pool(name="att", bufs=3))
            mm_ps = mctx.enter_context(tc.tile_pool(name="mmps", bufs=2, space="PSUM"))
            sc_ps = mctx.enter_context(tc.tile_pool(name="scps", bufs=2, space="PSUM"))
            dd_ps = mctx.enter_context(tc.tile_pool(name="ddps", bufs=2, space="PSUM"))

            def do_scores(b, o1T, dsc, after=None):
                ssc = sc_ps.tile([1, S], F32)
                for hk in range(MH):
                    mm = nc.tensor.matmul(
                        ssc[:],
                        dsc[:, 2 + hk : 3 + hk],
                        o1T[:, hk * S : (hk + 1) * S],
                        start=(hk == 0),
                        stop=(hk == MH - 1),
                    )
                    if after is not None:
                        # keep PE from stalling: order these matvecs after the
                        # next example's dense matmuls (order-only edge)
                        tile.add_dep_helper(
                            mm.ins, after.ins, sync=False,
                            reason="pipeline scores behind next example's mlp",
                        )
                sdd = dd_ps.tile([1, 1], F32)
                for hk in range(MH):
                    mm = nc.tensor.matmul(
                        sdd[:],
                        dsc[:, 2 + hk : 3 + hk],
                        dsc[:, hk : hk + 1],
                        start=(hk == 0),
                        stop=(hk == MH - 1),
                    )
                    if after is not None:
                        tile.add_dep_helper(
                            mm.ins, after.ins, sync=False,
                            reason="pipeline scores behind next example's mlp",
                        )
                # softmax on partition 0, straight from PSUM; no max-
                # subtraction (scores are O(25), far inside fp32 exp range)
                att = apool.tile([1, S], F32)
                s1 = apool.tile([1, 1], F32, name="s1")
                nc.scalar.activation(att[:], ssc[:], AF.Exp, accum_out=s1[:])
                edd = apool.tile([1, 1], F32, name="edd")
                nc.scalar.activation(edd[:], sdd[:], AF.Exp)
                stot = apool.tile([1, 1], F32, name="stot")
                nc.vector.tensor_add(stot[:], s1[:], edd[:])
                rs = apool.tile([1, 1], F32, name="rs")
                nc.vector.reciprocal(rs[:], stot[:])
                nc.vector.tensor_scalar_mul(att[:], att[:], rs[:])
                # SWDGE: keeps the blocking wait off the ACT/SP sequencers
                nc.gpsimd.dma_start(out[b : b + 1, 0:S], att[:])

            NCH = 3               # first example streams in chunk-tiles
            KPC = KV // NCH       # k-chunks per tile
            prev = None
            for b in range(BL):
                o1T = o1pool.tile([128, MH * S], F16)
                for i, xt in enumerate((x1t, x2t)):
                    xt_v = xt[b].rearrange("(k p) s -> p k s", p=128)
                    chunked = b == 0
                    if chunked:
                        xts = []
                        for c in range(NCH):
                            xc = xpool.tile([128, KPC * S], F16, name="xc",
                                            tag="xt_sb")
                            nc.sync.dma_start(
                                xc[:].rearrange("p (k s) -> p k s", k=KPC),
                                xt_v[:, c * KPC : (c + 1) * KPC, :],
                            )
                            xts.append(xc)
                    else:
                        xt_sb = xpool.tile([128, KV * S], F16, tag="xt_sb")
                        nc.sync.dma_start(
                            xt_sb[:].rearrange("p (k s) -> p k s", k=KV), xt_v
                        )
                    if b == 0 and i == 0:
                        _mlpb_dma()
                    if b == 1 and i == 0:
                        _late_const_dmas()
                    pss = [
                        mm_ps.tile([128, S], F32, tag=f"ps{m}", name=f"ps{m}")
                        for m in range(MH)
                    ]
                    for k in range(KV):
                        rhs = (
                            xts[k // KPC][:, (k % KPC) * S : (k % KPC + 1) * S]
                            if chunked
                            else xt_sb[:, k * S : (k + 1) * S]
                        )
                        for m in range(MH):
                            last_mm = nc.tensor.matmul(
                                pss[m][:],
                                wt_tiles[k][:, m * 128 : (m + 1) * 128],
                                rhs,
                                start=(k == 0),
                                stop=(k == KV - 1),
                            )
                    for m in range(MH):
                        kc = i * MH + m
                        if i == 0:
                            dst = o1T[:, m * S : (m + 1) * S]
                        else:
                            o2scr = o2pool.tile([128, S], F32)
                            dst = o2scr[:]
                        nc.scalar.activation(
                            dst,
                            pss[m][:],
                            AF.Relu,
                            bias=mlpb_sb[:, m : m + 1],
                            accum_out=docs_all[:, b * 4 + kc : b * 4 + kc + 1],
                        )

                if prev is not None:
                    do_scores(*prev, after=last_mm)
                # per-example scaled doc vectors: [o1d0, o1d1, o2d0, o2d1]
                dsc = dpool.tile([128, 4], F16)
                nc.vector.tensor_scalar_mul(
                    dsc[:], docs_all[:, b * 4 : b * 4 + 4], 1.0 / S
                )
                prev = (b, o1T, dsc)
            do_scores(*prev)

        # ---- head (batched over the BL examples) ----
        with ExitStack() as hctx:
            hpool = hctx.enter_context(tc.tile_pool(name="head", bufs=2))
            h_ps = hctx.enter_context(tc.tile_pool(name="hps", bufs=2, space="PSUM"))
            o_ps = hctx.enter_context(tc.tile_pool(name="ops", bufs=1, space="PSUM"))
            spool = hctx.enter_context(tc.tile_pool(name="smax", bufs=1))

            docs_sc = hpool.tile([128, 4 * BL], F32)
            nc.vector.tensor_scalar_mul(docs_sc[:], docs_all[:], 1.0 / S)
            docs_v = docs_sc[:].rearrange("p (b k) -> p k b", k=4)

            h_sb = hpool.tile([128, MH * BL], F32)
            for m in range(MH):
                ph = h_ps.tile([128, BL], F32)
                for kc in range(4):
                    nc.tensor.matmul(
                        ph[:],
                        fdwt_sb[:, kc * H + m * 128 : kc * H + (m + 1) * 128],
                        docs_v[:, kc, :],
                        start=(kc == 0),
                        stop=(kc == 3),
                    )
                nc.scalar.activation(
                    h_sb[:, m * BL : (m + 1) * BL],
                    ph[:],
                    AF.Relu,
                    bias=fdb_sb[:, m : m + 1],
                )
            po = o_ps.tile([1, BL], F32)
            for m in range(MH):
                nc.tensor.matmul(
                    po[:],
                    ffwt_sb[:, m : m + 1],
                    h_sb[:, m * BL : (m + 1) * BL],
                    start=(m == 0),
                    stop=(m == MH - 1),
                )
            # sigmoid(x) = 1/(1+exp(-x)) — stays in the Exp table set
            sig_row = hpool.tile([1, BL], F32)
            nc.scalar.activation(sig_row[:], po[:], AF.Exp,
                                 bias=nffb_sb[0:1, 0:1], scale=-1.0)
            nc.vector.tensor_scalar_add(sig_row[:], sig_row[:], 1.0)
            nc.vector.reciprocal(sig_row[:], sig_row[:])

            # final output column: out[:, S] = sigmoid head values
            nc.gpsimd.dma_start(
                out[:, S : S + 1],
                sig_row[0:1, :].rearrange("o (b s) -> o b s", b=BL),
            )


_NC_CACHE = None


def _get_nc():
    global _NC_CACHE
    if _NC_CACHE is None:
        nc = bacc.Bacc("TRN2", target_bir_lowering=False, debug=False,
                       num_devices=NCORES)
        with tile.TileContext(nc) as tc:
            _build_kernel(tc)
        nc.compile()
        _NC_CACHE = nc
    return _NC_CACHE


def kernel(output_1, output_2, mlp_w, mlp_b, fd_w, fd_b, ff_w, ff_b):
    output_1 = np.asarray(output_1, dtype=np.float32)
    output_2 = np.asarray(output_2, dtype=np.float32)
    mlp_w = np.asarray(mlp_w, dtype=np.float32)
    mlp_b = np.asarray(mlp_b, dtype=np.float32)
    fd_w = np.asarray(fd_w, dtype=np.float32)
    fd_b = np.asarray(fd_b, dtype=np.float32)
    ff_w = np.asarray(ff_w, dtype=np.float32)
    ff_b = np.asarray(ff_b, dtype=np.float32)

    # shard over batch, pre-transpose to [V, S]
    x1t = np.ascontiguousarray(
        output_1.reshape(NCORES, BL, S, V).transpose(0, 1, 3, 2)
    ).astype(np.float16)
    x2t = np.ascontiguousarray(
        output_2.reshape(NCORES, BL, S, V).transpose(0, 1, 3, 2)
    ).astype(np.float16)
    wt = np.ascontiguousarray(mlp_w.T).astype(np.float16)  # [V, H]
    mlpb = np.ascontiguousarray(mlp_b.reshape(H, 1))
    fdwt = np.ascontiguousarray(fd_w.T)                   # [2H, H]
    fdb = np.ascontiguousarray(fd_b.reshape(H, 1))
    ffwt = np.ascontiguousarray(ff_w.T)                   # [H, 1]
    ffb = np.ascontiguousarray(ff_b.reshape(1, 1))

    in_maps = [
        dict(x1t=x1t[c], x2t=x2t[c], wt=wt, mlp_b=mlpb, fdwt=fdwt,
             fd_b=fdb, ffwt=ffwt, ff_b=ffb)
        for c in range(NCORES)
    ]
    global _LAST_IN_MAPS
    _LAST_IN_MAPS = in_maps
    nc = _get_nc()
    res = bass_utils.run_bass_kernel_spmd(nc, in_maps, core_ids=list(range(NCORES)))
    att = np.concatenate([res.results[c]["out"] for c in range(NCORES)], axis=0)
    return np.ascontiguousarray(att.T)  # [S+1, B]



# revision 57
# speedup vs baseline: 1.9892x; 1.9892x over previous
"""Bass/Trainium2 kernel for nn_Bert_coss (8-core data-parallel over batch).

Computation (per example):
  o1 = relu(X1 @ W.T + b)            [S, H]
  o2 = relu(X2 @ W.T + b)            [S, H]
  o1_doc, o2_doc = mean over S       [H]
  out = sigmoid(relu(concat(o1_doc, o2_doc) @ fd_w.T + fd_b) @ ff_w.T + ff_b)
  scores[s] = o1e[s] . o2_doc   (o1e = o1 ++ o1_doc row), s in 0..S
  att = softmax(scores); output rows 0..S-1 = att[0:S], row S = out.

Key algorithmic simplification: the reference's full [S+1,S+1] co-attention
einsum is only consumed through its last column, so only S+1 dot products
against o2_doc are needed.

Precision split: the X1 branch feeds the softmax scores directly and stays
fp16; the X2 branch is only consumed through o2_doc (a mean over S=512 rows,
which averages per-element quantization error down ~sqrt(S)), so it runs in
fp8 e4m3 with DoubleRow perf mode (two 128-deep k-tiles per pass). The fp8
weight copy is pre-scaled by W8SC to clear the e4m3 subnormal range; the
activation eviction applies 1/W8SC before the bias.

Layouts are host-prepared to match SBUF exactly: X tiles land as
[128, KV*S] with one contiguous 6KB (fp16) / 3KB (fp8) line per partition,
so every DMA moves 128 long contiguous descriptors.
"""

import sys

for _p in ("/opt/trn_rl_repo",):
    if _p not in sys.path:
        sys.path.append(_p)

import numpy as np
import ml_dtypes
from contextlib import ExitStack

import concourse.bass as bass
import concourse.tile as tile
from concourse import bacc, mybir
from concourse import bass_utils

B, S, V, H = 64, 512, 768, 256
NCORES = 8
BL = B // NCORES        # examples per core
KV = V // 128           # contraction chunks for the fp16 mlp matmul
KP = KV // 2            # fp8 DoubleRow k-pair count
MH = H // 128           # output-partition chunks of H
W8SC = 64.0             # fp8 weight pre-scale (undone in the ACT eviction)

F32 = mybir.dt.float32
F16 = mybir.dt.float16
F8 = mybir.dt.float8e4
AF = mybir.ActivationFunctionType
DR = mybir.MatmulPerfMode.DoubleRow

# which DMA queue carries the x2 stream. "scalar" (the ACT queue): HW DMA
# bandwidth is capped per queue, so splitting the x1 (SP) and x2 (ACT)
# streams across two queues overlaps their transfers.
X2_QUEUE = "scalar"


def _build_kernel(tc):
    nc = tc.nc
    x1t = nc.dram_tensor("x1t", [BL, 128, KV * S], F16, kind="ExternalInput").ap()
    x2q = nc.dram_tensor("x2q", [BL, 128, KV * S], F8, kind="ExternalInput").ap()
    wt = nc.dram_tensor("wt", [128, KV * H], F16, kind="ExternalInput").ap()
    w8 = nc.dram_tensor("w8", [128, KP * 2 * H], F8, kind="ExternalInput").ap()
    mlp_b = nc.dram_tensor("mlp_b", [H, 1], F32, kind="ExternalInput").ap()
    fdwt = nc.dram_tensor("fdwt", [2 * H, H], F32, kind="ExternalInput").ap()
    fd_b = nc.dram_tensor("fd_b", [H, 1], F32, kind="ExternalInput").ap()
    ffwt = nc.dram_tensor("ffwt", [H, 1], F32, kind="ExternalInput").ap()
    ff_b = nc.dram_tensor("ff_b", [1, 1], F32, kind="ExternalInput").ap()
    out = nc.dram_tensor("out", [BL, S + 1], F32, kind="ExternalOutput").ap()
    _build_body(tc, x1t, x2q, wt, w8, mlp_b, fdwt, fd_b, ffwt, ff_b, out)


def _build_body(tc, x1t, x2q, wt, w8, mlp_b, fdwt, fd_b, ffwt, ff_b, out):
    nc = tc.nc
    with ExitStack() as ctx:
        const = ctx.enter_context(tc.tile_pool(name="const", bufs=1))

        # PE p-state warmup: the tensor engine clock ramps 0.65->1.2->2.4 GHz
        # over ~3us of sustained use. Start it on a memset tile at t~0 so the
        # ramp overlaps the input-DMA ramp instead of the first real matmuls.
        warm_sb = const.tile([1, 1], F16)
        nc.vector.memset(warm_sb[:], 0.0)

        # weight chunks DMA'd per-k into one tile so the k=0 matmul only
        # depends on the first small DMA (subtile deps). k=0 and the fp8
        # weights go on the SP queue interleaved with the first x1 chunks:
        # the ACT queue opens with a 1.3us LoadActFuncSet that would delay
        # them otherwise. k>=1 rides the ACT queue behind the table load.
        wt_sb = const.tile([128, KV * H], F16)
        w8_sb = const.tile([128, KP * 2 * H], F8)

        def _wt0_dmas():
            nc.sync.dma_start(wt_sb[:, 0:H], wt[:, 0:H])

        def _w8_dma():
            nc.sync.dma_start(w8_sb[:], w8[:, :])

        def _wt_rest_dmas():
            for k in range(1, KV):
                nc.scalar.dma_start(
                    wt_sb[:, k * H : (k + 1) * H], wt[:, k * H : (k + 1) * H]
                )

        mlpb_sb = const.tile([128, MH], F32)
        negb_sb = const.tile([128, MH], F32)
        bsc_sb = const.tile([128, MH], F16)
        bias4_sb = const.tile([128, 4], F32)
        fdwt_sb = const.tile([128, 4 * H], F32)
        fdb_sb = const.tile([128, MH], F32)
        ffwt_sb = const.tile([128, MH], F32)
        ffb_sb = const.tile([1, 1], F32)
        nffb_sb = const.tile([1, 1], F32)
        expwarm = const.tile([1, 1], F32)
        # dummy Exp so the ACT table set loads during the DMA ramp instead of
        # on the end-of-kernel critical path
        nc.scalar.activation(expwarm[:], warm_sb[0:1, 0:1], AF.Exp, scale=0.0)

        def _mlpb_dma():
            nc.scalar.dma_start(
                mlpb_sb[:].rearrange("p (m o) -> p m o", m=MH),
                mlp_b.rearrange("(m p) o -> p m o", p=128),
            )
            # Most relu evictions use relu(y+b) = max(y,-b)+b on DVE
            # (keeping ACT mostly free for the x2 DMAs + softmax exps); the
            # X2 m=1 eviction runs relu-form on ACT. The dsc/head paths
            # re-add b via bias4 = (b0, b1, b0, 0); the scores pick their
            # correction c=b.o2doc up as the exp bias.
            nc.vector.tensor_scalar_mul(negb_sb[:], mlpb_sb[:], -1.0)
            nc.vector.tensor_scalar_add(bsc_sb[:], mlpb_sb[:], 0.0)
            nc.vector.tensor_scalar_add(bias4_sb[:, 0:2], mlpb_sb[:, 0:MH], 0.0)
            nc.vector.tensor_scalar_add(bias4_sb[:, 2:3], mlpb_sb[:, 0:1], 0.0)
            nc.vector.memset(bias4_sb[:, 3:4], 0.0)

        def _late_const_dmas():
            # parameters only needed by the end-of-kernel head; SWDGE on the
            # near-idle Pool queue, keeping the ACT queue for the x2 stream
            nc.gpsimd.dma_start(
                fdwt_sb[:].rearrange("p (k h) -> p k h", k=4),
                fdwt.rearrange("(k p) h -> p k h", p=128),
            )
            nc.gpsimd.dma_start(
                fdb_sb[:].rearrange("p (m o) -> p m o", m=MH),
                fd_b.rearrange("(m p) o -> p m o", p=128),
            )
            nc.gpsimd.dma_start(
                ffwt_sb[:].rearrange("p (m o) -> p m o", m=MH),
                ffwt.rearrange("(m p) o -> p m o", p=128),
            )
            nc.gpsimd.dma_start(ffb_sb[:], ff_b[:, :])
            nc.vector.tensor_scalar_mul(nffb_sb[:], ffb_sb[:], -1.0)

        # doc-vector raw sums; column b*4 + kc, kc in (o1m0, o1m1, o2m0, o2m1)
        docs_all = const.tile([128, 4 * BL], F32)

        with ExitStack() as mctx:
            x1pool = mctx.enter_context(tc.tile_pool(name="x1", bufs=3))
            x2pool = mctx.enter_context(tc.tile_pool(name="x2", bufs=3))
            o1pool = mctx.enter_context(tc.tile_pool(name="o1", bufs=2))
            o2pool = mctx.enter_context(tc.tile_pool(name="o2", bufs=2))
            dpool = mctx.enter_context(tc.tile_pool(name="docs", bufs=2))
            apool = mctx.enter_context(tc.tile_pool(name="att", bufs=3))
            # 8 PSUM banks: ps0 x2 + ps1 x1 (its Pool evict returns the bank
            # fast) + x2 pair + sc x2 + dd = 8
            mm_ps = mctx.enter_context(tc.tile_pool(name="mmps", bufs=2, space="PSUM"))
            x2_ps = mctx.enter_context(tc.tile_pool(name="x2ps", bufs=1, space="PSUM"))
            sc_ps = mctx.enter_context(tc.tile_pool(name="scps", bufs=2, space="PSUM"))
            dd_ps = mctx.enter_context(tc.tile_pool(name="ddps", bufs=1, space="PSUM"))

            # warmup matmul train (each [1,1]: ~instruction overhead only);
            # scribbles into the dd bank, overwritten by the first real
            # start=True accumulation
            wps = dd_ps.tile([1, 1], F32, tag="dd")
            for _ in range(24):
                nc.tensor.matmul(wps[:], warm_sb[:], warm_sb[:], start=True,
                                 stop=True)

            w8_v = w8_sb[:].rearrange("p (kk i h) -> p kk i h", kk=KP, i=2)

            def do_scores(b, o1T, dsc, last=False):
                # col 0: dd = o1doc.o2doc (exact); col 1: c = b.o2doc, the
                # correction for o1T holding relu(.+b)-b from the max-form
                # evictions — applied as the exp bias below. Emitted before
                # the ssc matvecs so the edd exp clears ACT first.
                sdd = dd_ps.tile([1, 2], F32, tag="dd")
                for hk in range(MH):
                    nc.tensor.matmul(
                        sdd[:, 0:1],
                        dsc[:, 2 + hk : 3 + hk],
                        dsc[:, hk : hk + 1],
                        start=(hk == 0),
                        stop=(hk == MH - 1),
                    )
                for hk in range(MH):
                    nc.tensor.matmul(
                        sdd[:, 1:2],
                        bsc_sb[:, hk : hk + 1],
                        dsc[:, 2 + hk : 3 + hk],
                        start=(hk == 0),
                        stop=(hk == MH - 1),
                    )
                edd = apool.tile([1, 1], F32, name="edd")
                nc.scalar.activation(edd[:], sdd[:, 0:1], AF.Exp)
                csb = apool.tile([1, 1], F32, name="csb")
                nc.vector.tensor_scalar_add(csb[:], sdd[:, 1:2], 0.0)
                ssc = sc_ps.tile([1, S], F32)
                for hk in range(MH):
                    nc.tensor.matmul(
                        ssc[:],
                        dsc[:, 2 + hk : 3 + hk],
                        o1T[:, hk * S : (hk + 1) * S],
                        start=(hk == 0),
                        stop=(hk == MH - 1),
                    )
                # softmax on partition 0, straight from PSUM; no max-
                # subtraction (scores are O(25), far inside fp32 exp range)
                att = apool.tile([1, S], F32)
                s1 = apool.tile([1, 1], F32, name="s1")
                nc.scalar.activation(att[:], ssc[:], AF.Exp, accum_out=s1[:],
                                     bias=csb[:])
                stot = apool.tile([1, 1], F32, name="stot")
                nc.vector.tensor_add(stot[:], s1[:], edd[:])
                rs = apool.tile([1, 1], F32, name="rs")
                nc.vector.reciprocal(rs[:], stot[:])
                nc.vector.tensor_scalar_mul(att[:], att[:], rs[:])
                if last:
                    # end of kernel: SP is idle and HWDGE avoids the SWDGE
                    # ring-drain cost on the final transfer
                    nc.sync.dma_start(out[b : b + 1, 0:S], att[:])
                else:
                    # SWDGE: keeps the blocking wait off the ACT/SP sequencers
                    nc.gpsimd.dma_start(out[b : b + 1, 0:S], att[:])

            def do_x2(b, x2_sb, o1T):
                """fp8 DoubleRow matmuls + doc-sum evictions + dsc + scores
                for example b (runs one example behind the X1 stream so the
                short X2 burst and the score matvecs fill PE time while the
                next x1 DMA lands)."""
                x2_v = x2_sb[:].rearrange("p (kk i s) -> p kk i s", kk=KP, i=2)
                ps2 = [
                    x2_ps.tile([128, S], F32, tag=f"xp{m}", name=f"ps2{m}")
                    for m in range(MH)
                ]
                for kk in range(KP):
                    for m in range(MH):
                        nc.tensor.matmul(
                            ps2[m][:],
                            w8_v[:, kk, :, m * 128 : (m + 1) * 128],
                            x2_v[:, kk, :, :],
                            start=(kk == 0),
                            stop=(kk == KP - 1),
                            perf_mode=DR,
                        )
                # X2 doc sums: m=0 fused sum_s max(psum/W8SC, -b) on DVE,
                # m=1 relu-form on ACT (GPSIMD cannot read PSUM, so the two
                # PSUM-consuming engines split the work)
                o2scr = o2pool.tile([128, S], F16, name="o2scr0", tag="o2scr0")
                nc.vector.scalar_tensor_tensor(
                    o2scr[:],
                    ps2[0][:],
                    1.0 / W8SC,
                    negb_sb[:, 0:1].to_broadcast((128, S)),
                    op0=mybir.AluOpType.mult,
                    op1=mybir.AluOpType.max,
                    accum_out=docs_all[:, b * 4 + 2 : b * 4 + 3],
                )
                o2scr2 = o2pool.tile([128, S], F16, name="o2scr1", tag="o2scr1")
                nc.scalar.activation(
                    o2scr2[:],
                    ps2[1][:],
                    AF.Relu,
                    bias=mlpb_sb[:, 1:2],
                    scale=1.0 / W8SC,
                    accum_out=docs_all[:, b * 4 + 3 : b * 4 + 4],
                )
                # per-example scaled doc vectors: [o1d0, o1d1, o2d0, o2d1]
                # (bias4 restores the +b the max-form eviction dropped)
                dsc = dpool.tile([128, 4], F16)
                nc.vector.scalar_tensor_tensor(
                    dsc[:],
                    docs_all[:, b * 4 : b * 4 + 4],
                    1.0 / S,
                    bias4_sb[:, 0:4],
                    op0=mybir.AluOpType.mult,
                    op1=mybir.AluOpType.add,
                )
                return (b, o1T, dsc)

            NCH = 3               # first example streams in chunks
            KPC = KV // NCH
            prev_o1T = None
            spend = None
            for b in range(BL):
                # DMA issue order matches PE consumption: wt0, x1[0] chunks,
                # x1[1], w8, then per iteration x2[b-1], x1[b+1]
                if b == 0:
                    _wt0_dmas()
                    x1_sb = x1pool.tile([128, KV * S], F16, tag="x1t_sb")
                    for c in range(NCH):
                        cl = c * KPC * S
                        ch = (c + 1) * KPC * S
                        nc.sync.dma_start(x1_sb[:, cl:ch], x1t[b][:, cl:ch])
                    x1n_sb = x1pool.tile([128, KV * S], F16, tag="x1t_sb")
                    for c in range(NCH):
                        cl = c * KPC * S
                        ch = (c + 1) * KPC * S
                        nc.sync.dma_start(x1n_sb[:, cl:ch], x1t[1][:, cl:ch])
                    _w8_dma()
                    _wt_rest_dmas()
                    _mlpb_dma()
                else:
                    x1_sb = x1n_sb
                    x2_sb = x2pool.tile([128, KV * S], F8, tag="x2t_sb")
                    getattr(nc, X2_QUEUE).dma_start(x2_sb[:], x2q[b - 1][:, :])
                    if b + 1 < BL:
                        # chunked so the next X1 block can start on the first
                        # half while the second is still in flight
                        x1n_sb = x1pool.tile([128, KV * S], F16, tag="x1t_sb")
                        hl = KV * S // 2
                        nc.sync.dma_start(x1n_sb[:, 0:hl], x1t[b + 1][:, 0:hl])
                        nc.sync.dma_start(x1n_sb[:, hl:], x1t[b + 1][:, hl:])
                    else:
                        x2l_sb = x2pool.tile([128, KV * S], F8, tag="x2t_sb")
                        getattr(nc, X2_QUEUE).dma_start(x2l_sb[:], x2q[b][:, :])
                if b == 1:
                    _late_const_dmas()

                o1T = o1pool.tile([128, MH * S], F16)
                pss = [
                    mm_ps.tile([128, S], F32, tag=f"ps{m}", name=f"ps{m}",
                               bufs=(2 if m == 0 else 1))
                    for m in range(MH)
                ]
                for k in range(KV):
                    for m in range(MH):
                        nc.tensor.matmul(
                            pss[m][:],
                            wt_sb[:, k * H + m * 128 : k * H + (m + 1) * 128],
                            x1_sb[:, k * S : (k + 1) * S],
                            start=(k == 0),
                            stop=(k == KV - 1),
                        )
                # evictions on DVE (max-form), not ACT: the softmax exps
                # must not queue behind these on the ACT sequencer
                for m, eng in ((0, nc.vector), (1, nc.vector)):
                    eng.scalar_tensor_tensor(
                        o1T[:, m * S : (m + 1) * S],
                        pss[m][:],
                        1.0,
                        negb_sb[:, m : m + 1].to_broadcast((128, S)),
                        op0=mybir.AluOpType.mult,
                        op1=mybir.AluOpType.max,
                        accum_out=docs_all[:, b * 4 + m : b * 4 + m + 1],
                    )

                # X2 runs one example behind X1; scores one further behind,
                # so their dsc dependency chain is long resolved when the PE
                # sequencer reaches the matvecs
                if b > 0:
                    pend = do_x2(b - 1, x2_sb, prev_o1T)
                    if spend is not None:
                        do_scores(*spend)
                    spend = pend
                prev_o1T = o1T

            headp = mctx.enter_context(tc.tile_pool(name="head", bufs=1))

            def do_head():
                # batched over the BL examples; PSUM comes from the mm_ps
                # tags (X1's banks, free once its last evictions complete)
                docs_sc = headp.tile([128, 4 * BL], F32)
                nc.vector.tensor_scalar_mul(docs_sc[:], docs_all[:], 1.0 / S)
                docs_v = docs_sc[:].rearrange("p (b k) -> p k b", k=4)
                # restore the +b the max-form evictions dropped (stripes
                # 0..2; stripe 3 came from the relu-form ACT eviction)
                for kc in range(3):
                    nc.vector.tensor_add(
                        docs_v[:, kc, :],
                        docs_v[:, kc, :],
                        mlpb_sb[:, kc % MH : kc % MH + 1].to_broadcast((128, BL)),
                    )

                h_sb = headp.tile([128, MH * BL], F32)
                for m in range(MH):
                    ph = mm_ps.tile([128, BL], F32, tag="ps0", name="ph", bufs=2)
                    for kc in range(4):
                        nc.tensor.matmul(
                            ph[:],
                            fdwt_sb[:, kc * H + m * 128 : kc * H + (m + 1) * 128],
                            docs_v[:, kc, :],
                            start=(kc == 0),
                            stop=(kc == 3),
                        )
                    nc.scalar.activation(
                        h_sb[:, m * BL : (m + 1) * BL],
                        ph[:],
                        AF.Relu,
                        bias=fdb_sb[:, m : m + 1],
                    )
                po = mm_ps.tile([1, BL], F32, tag="ps1", name="po", bufs=1)
                for m in range(MH):
                    nc.tensor.matmul(
                        po[:],
                        ffwt_sb[:, m : m + 1],
                        h_sb[:, m * BL : (m + 1) * BL],
                        start=(m == 0),
                        stop=(m == MH - 1),
                    )
                # sigmoid(x) = 1/(1+exp(-x)) — stays in the Exp table set
                sig_row = headp.tile([1, BL], F32)
                nc.scalar.activation(sig_row[:], po[:], AF.Exp,
                                     bias=nffb_sb[0:1, 0:1], scale=-1.0)
                nc.vector.tensor_scalar_add(sig_row[:], sig_row[:], 1.0)
                nc.vector.reciprocal(sig_row[:], sig_row[:])

                # final output column: out[:, S] = sigmoid head values
                nc.sync.dma_start(
                    out[:, S : S + 1],
                    sig_row[0:1, :].rearrange("o (b s) -> o b s", b=BL),
                )

            pend = do_x2(BL - 1, x2l_sb, prev_o1T)
            do_scores(*spend)
            do_head()
            do_scores(*pend, last=True)


_NC_CACHE = None


def _get_nc():
    global _NC_CACHE
    if _NC_CACHE is None:
        nc = bacc.Bacc("TRN2", target_bir_lowering=False, debug=False,
                       num_devices=NCORES)
        with tile.TileContext(nc) as tc:
            _build_kernel(tc)
        nc.compile()
        _NC_CACHE = nc
    return _NC_CACHE


def make_in_maps(output_1, output_2, mlp_w, mlp_b, fd_w, fd_b, ff_w, ff_b):
    output_1 = np.asarray(output_1, dtype=np.float32)
    output_2 = np.asarray(output_2, dtype=np.float32)
    mlp_w = np.asarray(mlp_w, dtype=np.float32)
    mlp_b = np.asarray(mlp_b, dtype=np.float32)
    fd_w = np.asarray(fd_w, dtype=np.float32)
    fd_b = np.asarray(fd_b, dtype=np.float32)
    ff_w = np.asarray(ff_w, dtype=np.float32)
    ff_b = np.asarray(ff_b, dtype=np.float32)

    # shard over batch; lay out [core, b, p, k*S+s] so each SBUF partition
    # line is one contiguous DRAM read
    x1t = np.ascontiguousarray(
        output_1.reshape(NCORES, BL, S, KV, 128).transpose(0, 1, 4, 3, 2)
    ).reshape(NCORES, BL, 128, KV * S).astype(np.float16)
    x2q = np.ascontiguousarray(
        output_2.reshape(NCORES, BL, S, KV, 128).transpose(0, 1, 4, 3, 2)
    ).reshape(NCORES, BL, 128, KV * S).astype(ml_dtypes.float8_e4m3)
    # wt[p, k*H+h] = W[h, k*128+p]
    wt = np.ascontiguousarray(
        mlp_w.T.reshape(KV, 128, H).transpose(1, 0, 2)
    ).reshape(128, KV * H).astype(np.float16)
    # w8[p, kk*2H + i*H + h] = W8SC * W[h, (2kk+i)*128+p]
    w8 = np.ascontiguousarray(
        (mlp_w.T * W8SC).reshape(KP, 2, 128, H).transpose(2, 0, 1, 3)
    ).reshape(128, KP * 2 * H).astype(ml_dtypes.float8_e4m3)
    mlpb = np.ascontiguousarray(mlp_b.reshape(H, 1))
    fdwt = np.ascontiguousarray(fd_w.T)                   # [2H, H]
    fdb = np.ascontiguousarray(fd_b.reshape(H, 1))
    ffwt = np.ascontiguousarray(ff_w.T)                   # [H, 1]
    ffb = np.ascontiguousarray(ff_b.reshape(1, 1))

    return [
        dict(x1t=x1t[c], x2q=x2q[c], wt=wt, w8=w8, mlp_b=mlpb, fdwt=fdwt,
             fd_b=fdb, ffwt=ffwt, ff_b=ffb)
        for c in range(NCORES)
    ]


def kernel(**inputs):
    in_maps = make_in_maps(**inputs)
    global _LAST_IN_MAPS
    _LAST_IN_MAPS = in_maps
    nc = _get_nc()
    res = bass_utils.run_bass_kernel_spmd(nc, in_maps, core_ids=list(range(NCORES)))
    att = np.concatenate([res.results[c]["out"] for c in range(NCORES)], axis=0)
    return np.ascontiguousarray(att.T)  # [S+1, B]


# revision 61
# speedup vs baseline: 1.9929x; 1.0019x over previous
"""Bass/Trainium2 kernel for nn_Bert_coss (8-core data-parallel over batch).

Computation (per example):
  o1 = relu(X1 @ W.T + b)            [S, H]
  o2 = relu(X2 @ W.T + b)            [S, H]
  o1_doc, o2_doc = mean over S       [H]
  out = sigmoid(relu(concat(o1_doc, o2_doc) @ fd_w.T + fd_b) @ ff_w.T + ff_b)
  scores[s] = o1e[s] . o2_doc   (o1e = o1 ++ o1_doc row), s in 0..S
  att = softmax(scores); output rows 0..S-1 = att[0:S], row S = out.

Key algorithmic simplification: the reference's full [S+1,S+1] co-attention
einsum is only consumed through its last column, so only S+1 dot products
against o2_doc are needed.

Precision split: the X1 branch feeds the softmax scores directly and stays
fp16; the X2 branch is only consumed through o2_doc (a mean over S=512 rows,
which averages per-element quantization error down ~sqrt(S)), so it runs in
fp8 e4m3 with DoubleRow perf mode (two 128-deep k-tiles per pass). The fp8
weight copy is pre-scaled by W8SC to clear the e4m3 subnormal range; the
activation eviction applies 1/W8SC before the bias.

Layouts are host-prepared to match SBUF exactly: X tiles land as
[128, KV*S] with one contiguous 6KB (fp16) / 3KB (fp8) line per partition,
so every DMA moves 128 long contiguous descriptors.
"""

import sys

for _p in ("/opt/trn_rl_repo",):
    if _p not in sys.path:
        sys.path.append(_p)

import numpy as np
import ml_dtypes
from contextlib import ExitStack

import concourse.bass as bass
import concourse.tile as tile
from concourse import bacc, mybir
from concourse import bass_utils

B, S, V, H = 64, 512, 768, 256
NCORES = 8
BL = B // NCORES        # examples per core
KV = V // 128           # contraction chunks for the fp16 mlp matmul
KP = KV // 2            # fp8 DoubleRow k-pair count
MH = H // 128           # output-partition chunks of H
W8SC = 64.0             # fp8 weight pre-scale (undone in the ACT eviction)

F32 = mybir.dt.float32
F16 = mybir.dt.float16
F8 = mybir.dt.float8e4
AF = mybir.ActivationFunctionType
DR = mybir.MatmulPerfMode.DoubleRow

# which DMA queue carries the x2 stream. "scalar" (the ACT queue): HW DMA
# bandwidth is capped per queue, so splitting the x1 (SP) and x2 (ACT)
# streams across two queues overlaps their transfers.
X2_QUEUE = "scalar"


def _build_kernel(tc):
    nc = tc.nc
    x1t = nc.dram_tensor("x1t", [BL, 128, KV * S], F16, kind="ExternalInput").ap()
    x2q = nc.dram_tensor("x2q", [BL, 128, KV * S], F8, kind="ExternalInput").ap()
    wt = nc.dram_tensor("wt", [128, KV * H], F16, kind="ExternalInput").ap()
    w8 = nc.dram_tensor("w8", [128, KP * 2 * H], F8, kind="ExternalInput").ap()
    mlp_b = nc.dram_tensor("mlp_b", [H, 1], F32, kind="ExternalInput").ap()
    fdwt = nc.dram_tensor("fdwt", [2 * H, H], F32, kind="ExternalInput").ap()
    fd_b = nc.dram_tensor("fd_b", [H, 1], F32, kind="ExternalInput").ap()
    ffwt = nc.dram_tensor("ffwt", [H, 1], F32, kind="ExternalInput").ap()
    ff_b = nc.dram_tensor("ff_b", [1, 1], F32, kind="ExternalInput").ap()
    out = nc.dram_tensor("out", [BL, S + 1], F32, kind="ExternalOutput").ap()
    _build_body(tc, x1t, x2q, wt, w8, mlp_b, fdwt, fd_b, ffwt, ff_b, out)


def _build_body(tc, x1t, x2q, wt, w8, mlp_b, fdwt, fd_b, ffwt, ff_b, out):
    nc = tc.nc
    with ExitStack() as ctx:
        const = ctx.enter_context(tc.tile_pool(name="const", bufs=1))

        # PE p-state warmup: the tensor engine clock ramps 0.65->1.2->2.4 GHz
        # over ~3us of sustained use. Start it on a memset tile at t~0 so the
        # ramp overlaps the input-DMA ramp instead of the first real matmuls.
        warm_sb = const.tile([1, 1], F16)
        nc.vector.memset(warm_sb[:], 0.0)

        # weight chunks DMA'd per-k into one tile so the k=0 matmul only
        # depends on the first small DMA (subtile deps). k=0 and the fp8
        # weights go on the SP queue interleaved with the first x1 chunks:
        # the ACT queue opens with a 1.3us LoadActFuncSet that would delay
        # them otherwise. k>=1 rides the ACT queue behind the table load.
        wt_sb = const.tile([128, KV * H], F16)
        w8_sb = const.tile([128, KP * 2 * H], F8)

        def _wt0_dmas():
            nc.sync.dma_start(wt_sb[:, 0:H], wt[:, 0:H])

        def _w8_dma():
            nc.sync.dma_start(w8_sb[:], w8[:, :])

        def _wt_rest_dmas():
            for k in range(1, KV):
                nc.scalar.dma_start(
                    wt_sb[:, k * H : (k + 1) * H], wt[:, k * H : (k + 1) * H]
                )

        mlpb_sb = const.tile([128, MH], F32)
        negb_sb = const.tile([128, MH], F32)
        bsc_sb = const.tile([128, MH], F16)
        bias4_sb = const.tile([128, 4], F32)
        fdwt_sb = const.tile([128, 4 * H], F32)
        fdb_sb = const.tile([128, MH], F32)
        ffwt_sb = const.tile([128, MH], F32)
        ffb_sb = const.tile([1, 1], F32)
        nffb_sb = const.tile([1, 1], F32)
        expwarm = const.tile([1, 1], F32)
        # dummy Exp so the ACT table set loads during the DMA ramp instead of
        # on the end-of-kernel critical path
        nc.scalar.activation(expwarm[:], warm_sb[0:1, 0:1], AF.Exp, scale=0.0)

        def _mlpb_dma():
            nc.scalar.dma_start(
                mlpb_sb[:].rearrange("p (m o) -> p m o", m=MH),
                mlp_b.rearrange("(m p) o -> p m o", p=128),
            )
            # Most relu evictions use relu(y+b) = max(y,-b)+b on DVE
            # (keeping ACT mostly free for the x2 DMAs + softmax exps); the
            # X2 m=1 eviction runs relu-form on ACT. The dsc/head paths
            # re-add b via bias4 = (b0, b1, b0, 0); the scores pick their
            # correction c=b.o2doc up as the exp bias.
            nc.vector.tensor_scalar_mul(negb_sb[:], mlpb_sb[:], -1.0)
            nc.vector.tensor_scalar_add(bsc_sb[:], mlpb_sb[:], 0.0)
            nc.vector.tensor_scalar_add(bias4_sb[:, 0:2], mlpb_sb[:, 0:MH], 0.0)
            nc.vector.tensor_scalar_add(bias4_sb[:, 2:3], mlpb_sb[:, 0:1], 0.0)
            nc.vector.memset(bias4_sb[:, 3:4], 0.0)

        def _late_const_dmas():
            # parameters only needed by the end-of-kernel head; SWDGE on the
            # near-idle Pool queue, keeping the ACT queue for the x2 stream
            nc.gpsimd.dma_start(
                fdwt_sb[:].rearrange("p (k h) -> p k h", k=4),
                fdwt.rearrange("(k p) h -> p k h", p=128),
            )
            nc.gpsimd.dma_start(
                fdb_sb[:].rearrange("p (m o) -> p m o", m=MH),
                fd_b.rearrange("(m p) o -> p m o", p=128),
            )
            nc.gpsimd.dma_start(
                ffwt_sb[:].rearrange("p (m o) -> p m o", m=MH),
                ffwt.rearrange("(m p) o -> p m o", p=128),
            )
            nc.gpsimd.dma_start(ffb_sb[:], ff_b[:, :])
            nc.vector.tensor_scalar_mul(nffb_sb[:], ffb_sb[:], -1.0)

        # doc-vector raw sums; column b*4 + kc, kc in (o1m0, o1m1, o2m0, o2m1)
        docs_all = const.tile([128, 4 * BL], F32)

        with ExitStack() as mctx:
            x1pool = mctx.enter_context(tc.tile_pool(name="x1", bufs=4))
            x2pool = mctx.enter_context(tc.tile_pool(name="x2", bufs=4))
            o1pool = mctx.enter_context(tc.tile_pool(name="o1", bufs=3))
            o2pool = mctx.enter_context(tc.tile_pool(name="o2", bufs=3))
            dpool = mctx.enter_context(tc.tile_pool(name="docs", bufs=3))
            apool = mctx.enter_context(tc.tile_pool(name="att", bufs=4))
            # 8 PSUM banks: ps0 x2 + ps1 x1 (its Pool evict returns the bank
            # fast) + x2 pair + sc x2 + dd = 8
            mm_ps = mctx.enter_context(tc.tile_pool(name="mmps", bufs=2, space="PSUM"))
            x2_ps = mctx.enter_context(tc.tile_pool(name="x2ps", bufs=1, space="PSUM"))
            sc_ps = mctx.enter_context(tc.tile_pool(name="scps", bufs=2, space="PSUM"))
            dd_ps = mctx.enter_context(tc.tile_pool(name="ddps", bufs=1, space="PSUM"))

            # warmup matmul train (each [1,1]: ~instruction overhead only);
            # scribbles into the dd bank, overwritten by the first real
            # start=True accumulation
            wps = dd_ps.tile([1, 1], F32, tag="dd")
            for _ in range(24):
                nc.tensor.matmul(wps[:], warm_sb[:], warm_sb[:], start=True,
                                 stop=True)

            w8_v = w8_sb[:].rearrange("p (kk i h) -> p kk i h", kk=KP, i=2)

            def do_scores(b, o1T, dsc, last=False):
                # col 0: dd = o1doc.o2doc (exact); col 1: c = b.o2doc, the
                # correction for o1T holding relu(.+b)-b from the max-form
                # evictions — applied as the exp bias below. Emitted before
                # the ssc matvecs so the edd exp clears ACT first.
                sdd = dd_ps.tile([1, 2], F32, tag="dd")
                for hk in range(MH):
                    nc.tensor.matmul(
                        sdd[:, 0:1],
                        dsc[:, 2 + hk : 3 + hk],
                        dsc[:, hk : hk + 1],
                        start=(hk == 0),
                        stop=(hk == MH - 1),
                    )
                for hk in range(MH):
                    nc.tensor.matmul(
                        sdd[:, 1:2],
                        bsc_sb[:, hk : hk + 1],
                        dsc[:, 2 + hk : 3 + hk],
                        start=(hk == 0),
                        stop=(hk == MH - 1),
                    )
                edd = apool.tile([1, 1], F32, name="edd")
                nc.scalar.activation(edd[:], sdd[:, 0:1], AF.Exp)
                csb = apool.tile([1, 1], F32, name="csb")
                nc.vector.tensor_scalar_add(csb[:], sdd[:, 1:2], 0.0)
                ssc = sc_ps.tile([1, S], F32)
                for hk in range(MH):
                    nc.tensor.matmul(
                        ssc[:],
                        dsc[:, 2 + hk : 3 + hk],
                        o1T[:, hk * S : (hk + 1) * S],
                        start=(hk == 0),
                        stop=(hk == MH - 1),
                    )
                # softmax on partition 0, straight from PSUM; no max-
                # subtraction (scores are O(25), far inside fp32 exp range)
                att = apool.tile([1, S], F32)
                s1 = apool.tile([1, 1], F32, name="s1")
                nc.scalar.activation(att[:], ssc[:], AF.Exp, accum_out=s1[:],
                                     bias=csb[:])
                stot = apool.tile([1, 1], F32, name="stot")
                nc.vector.tensor_add(stot[:], s1[:], edd[:])
                rs = apool.tile([1, 1], F32, name="rs")
                nc.vector.reciprocal(rs[:], stot[:])
                nc.vector.tensor_scalar_mul(att[:], att[:], rs[:])
                if last:
                    # end of kernel: SP is idle and HWDGE avoids the SWDGE
                    # ring-drain cost on the final transfer
                    nc.sync.dma_start(out[b : b + 1, 0:S], att[:])
                else:
                    # SWDGE: keeps the blocking wait off the ACT/SP sequencers
                    nc.gpsimd.dma_start(out[b : b + 1, 0:S], att[:])

            def do_x2(b, x2_sb, o1T):
                """fp8 DoubleRow matmuls + doc-sum evictions + dsc + scores
                for example b (runs one example behind the X1 stream so the
                short X2 burst and the score matvecs fill PE time while the
                next x1 DMA lands)."""
                x2_v = x2_sb[:].rearrange("p (kk i s) -> p kk i s", kk=KP, i=2)
                ps2 = [
                    x2_ps.tile([128, S], F32, tag=f"xp{m}", name=f"ps2{m}")
                    for m in range(MH)
                ]
                for kk in range(KP):
                    for m in range(MH):
                        nc.tensor.matmul(
                            ps2[m][:],
                            w8_v[:, kk, :, m * 128 : (m + 1) * 128],
                            x2_v[:, kk, :, :],
                            start=(kk == 0),
                            stop=(kk == KP - 1),
                            perf_mode=DR,
                        )
                # X2 doc sums: m=0 fused sum_s max(psum/W8SC, -b) on DVE,
                # m=1 relu-form on ACT (GPSIMD cannot read PSUM, so the two
                # PSUM-consuming engines split the work)
                o2scr = o2pool.tile([128, S], F16, name="o2scr0", tag="o2scr0")
                nc.vector.scalar_tensor_tensor(
                    o2scr[:],
                    ps2[0][:],
                    1.0 / W8SC,
                    negb_sb[:, 0:1].to_broadcast((128, S)),
                    op0=mybir.AluOpType.mult,
                    op1=mybir.AluOpType.max,
                    accum_out=docs_all[:, b * 4 + 2 : b * 4 + 3],
                )
                o2scr2 = o2pool.tile([128, S], F16, name="o2scr1", tag="o2scr1")
                nc.scalar.activation(
                    o2scr2[:],
                    ps2[1][:],
                    AF.Relu,
                    bias=mlpb_sb[:, 1:2],
                    scale=1.0 / W8SC,
                    accum_out=docs_all[:, b * 4 + 3 : b * 4 + 4],
                )
                # per-example scaled doc vectors: [o1d0, o1d1, o2d0, o2d1]
                # (bias4 restores the +b the max-form eviction dropped)
                dsc = dpool.tile([128, 4], F16)
                nc.vector.scalar_tensor_tensor(
                    dsc[:],
                    docs_all[:, b * 4 : b * 4 + 4],
                    1.0 / S,
                    bias4_sb[:, 0:4],
                    op0=mybir.AluOpType.mult,
                    op1=mybir.AluOpType.add,
                )
                return (b, o1T, dsc)

            NCH = 3               # first example streams in chunks
            KPC = KV // NCH
            prev_o1T = None
            spend = None
            for b in range(BL):
                # DMA issue order matches PE consumption: wt0, x1[0] chunks,
                # x1[1], w8, then per iteration x2[b-1], x1[b+1]
                if b == 0:
                    _wt0_dmas()
                    x1_sb = x1pool.tile([128, KV * S], F16, tag="x1t_sb")
                    for c in range(NCH):
                        cl = c * KPC * S
                        ch = (c + 1) * KPC * S
                        nc.sync.dma_start(x1_sb[:, cl:ch], x1t[b][:, cl:ch])
                    x1n_sb = x1pool.tile([128, KV * S], F16, tag="x1t_sb")
                    for c in range(NCH):
                        cl = c * KPC * S
                        ch = (c + 1) * KPC * S
                        nc.sync.dma_start(x1n_sb[:, cl:ch], x1t[1][:, cl:ch])
                    _w8_dma()
                    _wt_rest_dmas()
                    _mlpb_dma()
                else:
                    x1_sb = x1n_sb
                    x2_sb = x2pool.tile([128, KV * S], F8, tag="x2t_sb")
                    h2 = KV * S // 2
                    getattr(nc, X2_QUEUE).dma_start(
                        x2_sb[:, 0:h2], x2q[b - 1][:, 0:h2])
                    nc.sync.dma_start(x2_sb[:, h2:], x2q[b - 1][:, h2:])
                    if b + 1 < BL:
                        # halves on two queues: PE starts on the SP half
                        # while SWDGE moves the second half concurrently
                        # (HW DMA bandwidth is capped per queue)
                        x1n_sb = x1pool.tile([128, KV * S], F16, tag="x1t_sb")
                        hl = KV * S // 2
                        nc.sync.dma_start(x1n_sb[:, 0:hl], x1t[b + 1][:, 0:hl])
                        nc.gpsimd.dma_start(x1n_sb[:, hl:], x1t[b + 1][:, hl:])
                    else:
                        x2l_sb = x2pool.tile([128, KV * S], F8, tag="x2t_sb")
                        getattr(nc, X2_QUEUE).dma_start(x2l_sb[:], x2q[b][:, :])
                if b == 1:
                    _late_const_dmas()

                o1T = o1pool.tile([128, MH * S], F16)
                pss = [
                    mm_ps.tile([128, S], F32, tag=f"ps{m}", name=f"ps{m}",
                               bufs=(2 if m == 0 else 1))
                    for m in range(MH)
                ]
                for k in range(KV):
                    for m in range(MH):
                        nc.tensor.matmul(
                            pss[m][:],
                            wt_sb[:, k * H + m * 128 : k * H + (m + 1) * 128],
                            x1_sb[:, k * S : (k + 1) * S],
                            start=(k == 0),
                            stop=(k == KV - 1),
                        )
                # evictions on DVE (max-form), not ACT: the softmax exps
                # must not queue behind these on the ACT sequencer
                for m, eng in ((0, nc.vector), (1, nc.vector)):
                    eng.scalar_tensor_tensor(
                        o1T[:, m * S : (m + 1) * S],
                        pss[m][:],
                        1.0,
                        negb_sb[:, m : m + 1].to_broadcast((128, S)),
                        op0=mybir.AluOpType.mult,
                        op1=mybir.AluOpType.max,
                        accum_out=docs_all[:, b * 4 + m : b * 4 + m + 1],
                    )

                # X2 runs one example behind X1; scores one further behind,
                # so their dsc dependency chain is long resolved when the PE
                # sequencer reaches the matvecs
                if b > 0:
                    pend = do_x2(b - 1, x2_sb, prev_o1T)
                    if spend is not None:
                        do_scores(*spend)
                    spend = pend
                prev_o1T = o1T

            headp = mctx.enter_context(tc.tile_pool(name="head", bufs=1))

            def do_head():
                # batched over the BL examples; PSUM comes from the mm_ps
                # tags (X1's banks, free once its last evictions complete)
                docs_sc = headp.tile([128, 4 * BL], F32)
                nc.vector.tensor_scalar_mul(docs_sc[:], docs_all[:], 1.0 / S)
                docs_v = docs_sc[:].rearrange("p (b k) -> p k b", k=4)
                # restore the +b the max-form evictions dropped (stripes
                # 0..2; stripe 3 came from the relu-form ACT eviction)
                for kc in range(3):
                    nc.vector.tensor_add(
                        docs_v[:, kc, :],
                        docs_v[:, kc, :],
                        mlpb_sb[:, kc % MH : kc % MH + 1].to_broadcast((128, BL)),
                    )

                h_sb = headp.tile([128, MH * BL], F32)
                for m in range(MH):
                    ph = mm_ps.tile([128, BL], F32, tag="ps0", name="ph", bufs=2)
                    for kc in range(4):
                        nc.tensor.matmul(
                            ph[:],
                            fdwt_sb[:, kc * H + m * 128 : kc * H + (m + 1) * 128],
                            docs_v[:, kc, :],
                            start=(kc == 0),
                            stop=(kc == 3),
                        )
                    nc.scalar.activation(
                        h_sb[:, m * BL : (m + 1) * BL],
                        ph[:],
                        AF.Relu,
                        bias=fdb_sb[:, m : m + 1],
                    )
                po = mm_ps.tile([1, BL], F32, tag="ps1", name="po", bufs=1)
                for m in range(MH):
                    nc.tensor.matmul(
                        po[:],
                        ffwt_sb[:, m : m + 1],
                        h_sb[:, m * BL : (m + 1) * BL],
                        start=(m == 0),
                        stop=(m == MH - 1),
                    )
                # sigmoid(x) = 1/(1+exp(-x)) — stays in the Exp table set
                sig_row = headp.tile([1, BL], F32)
                nc.scalar.activation(sig_row[:], po[:], AF.Exp,
                                     bias=nffb_sb[0:1, 0:1], scale=-1.0)
                nc.vector.tensor_scalar_add(sig_row[:], sig_row[:], 1.0)
                nc.vector.reciprocal(sig_row[:], sig_row[:])

                # final output column: out[:, S] = sigmoid head values
                nc.sync.dma_start(
                    out[:, S : S + 1],
                    sig_row[0:1, :].rearrange("o (b s) -> o b s", b=BL),
                )

            pend = do_x2(BL - 1, x2l_sb, prev_o1T)
            do_scores(*spend)
            do_head()
            do_scores(*pend, last=True)


_NC_CACHE = None


def _get_nc():
    global _NC_CACHE
    if _NC_CACHE is None:
        nc = bacc.Bacc("TRN2", target_bir_lowering=False, debug=False,
                       num_devices=NCORES)
        with tile.TileContext(nc) as tc:
            _build_kernel(tc)
        nc.compile()
        _NC_CACHE = nc
    return _NC_CACHE


def make_in_maps(output_1, output_2, mlp_w, mlp_b, fd_w, fd_b, ff_w, ff_b):
    output_1 = np.asarray(output_1, dtype=np.float32)
    output_2 = np.asarray(output_2, dtype=np.float32)
    mlp_w = np.asarray(mlp_w, dtype=np.float32)
    mlp_b = np.asarray(mlp_b, dtype=np.float32)
    fd_w = np.asarray(fd_w, dtype=np.float32)
    fd_b = np.asarray(fd_b, dtype=np.float32)
    ff_w = np.asarray(ff_w, dtype=np.float32)
    ff_b = np.asarray(ff_b, dtype=np.float32)

    # shard over batch; lay out [core, b, p, k*S+s] so each SBUF partition
    # line is one contiguous DRAM read
    x1t = np.ascontiguousarray(
        output_1.reshape(NCORES, BL, S, KV, 128).transpose(0, 1, 4, 3, 2)
    ).reshape(NCORES, BL, 128, KV * S).astype(np.float16)
    x2q = np.ascontiguousarray(
        output_2.reshape(NCORES, BL, S, KV, 128).transpose(0, 1, 4, 3, 2)
    ).reshape(NCORES, BL, 128, KV * S).astype(ml_dtypes.float8_e4m3)
    # wt[p, k*H+h] = W[h, k*128+p]
    wt = np.ascontiguousarray(
        mlp_w.T.reshape(KV, 128, H).transpose(1, 0, 2)
    ).reshape(128, KV * H).astype(np.float16)
    # w8[p, kk*2H + i*H + h] = W8SC * W[h, (2kk+i)*128+p]
    w8 = np.ascontiguousarray(
        (mlp_w.T * W8SC).reshape(KP, 2, 128, H).transpose(2, 0, 1, 3)
    ).reshape(128, KP * 2 * H).astype(ml_dtypes.float8_e4m3)
    mlpb = np.ascontiguousarray(mlp_b.reshape(H, 1))
    fdwt = np.ascontiguousarray(fd_w.T)                   # [2H, H]
    fdb = np.ascontiguousarray(fd_b.reshape(H, 1))
    ffwt = np.ascontiguousarray(ff_w.T)                   # [H, 1]
    ffb = np.ascontiguousarray(ff_b.reshape(1, 1))

    return [
        dict(x1t=x1t[c], x2q=x2q[c], wt=wt, w8=w8, mlp_b=mlpb, fdwt=fdwt,
             fd_b=fdb, ffwt=ffwt, ff_b=ffb)
        for c in range(NCORES)
    ]


def kernel(**inputs):
    in_maps = make_in_maps(**inputs)
    global _LAST_IN_MAPS
    _LAST_IN_MAPS = in_maps
    nc = _get_nc()
    res = bass_utils.run_bass_kernel_spmd(nc, in_maps, core_ids=list(range(NCORES)))
    att = np.concatenate([res.results[c]["out"] for c in range(NCORES)], axis=0)
    return np.ascontiguousarray(att.T)  # [S+1, B]
